# revision 2
# baseline (speedup 1.0000x reference)
"""Trainium2 Bass kernel for nn_CrossAttention_249108103802.

Math (per batch b, done on one NeuronCore; 8 cores data-parallel over B=8):
  q_s   = heads(x_s)                   (just column slices of x_s)
  k,v   = x_s @ W_s  split per head    -> never materialized; instead:
  ctx_s = softmax_d(scale * k^T v)     via Gram trick:
          k_h^T v_h = Wk_h^T (x^T x) Wv_h
  o1    = q1 @ blockdiag(ctx2), o2 = q2 @ blockdiag(ctx1)

Precision: bf16 operands everywhere on the PE with fp32 PSUM accumulation,
plus a mu*I split of the Gram matrix (its diagonal ~N=4096 would lose
~16 abs in bf16) with the mu*Wv^T Wk correction computed in fp32, and a
hi/lo bf16 split of A = Gc @ Wv. Measured end-to-end rel err ~4e-3.
"""
import sys

sys.path.insert(0, "/opt/trn_rl_repo")

import numpy as np

import concourse.bass as bass
import concourse.mybir as mybir
import concourse.tile as tile
from concourse import bacc
from concourse.bass_utils import run_bass_kernel_spmd
from concourse.masks import make_identity

B, N, C, H = 8, 4096, 512, 8
HD = C // H                    # 64
SCALE = HD ** -0.5             # 1/8
MU = float(N)                  # expected Gram diagonal
NT = N // 128                  # 32 row tiles
CB = C // 128                  # 4 feature blocks
BF = mybir.dt.bfloat16
F32 = mybir.dt.float32
AF = mybir.ActivationFunctionType


def build():
    nc = bacc.Bacc("TRN2", target_bir_lowering=False, debug=False, num_devices=8)
    x_d = [nc.declare_dram_parameter("x1", [N, C], F32, isOutput=False),
           nc.declare_dram_parameter("x2", [N, C], F32, isOutput=False)]
    w_d = [nc.declare_dram_parameter("W_kv1", [C, 2 * C], F32, isOutput=False),
           nc.declare_dram_parameter("W_kv2", [C, 2 * C], F32, isOutput=False)]
    o_d = [nc.declare_dram_parameter("o1", [N, C], BF, isOutput=True),
           nc.declare_dram_parameter("o2", [N, C], BF, isOutput=True)]

    with tile.TileContext(nc) as tc:
        with (
            tc.tile_pool(name="const", bufs=1) as constp,
            tc.tile_pool(name="wf", bufs=1) as wfp,
            tc.tile_pool(name="w", bufs=2) as wp,
            tc.tile_pool(name="x", bufs=4) as xp,
            tc.tile_pool(name="xt", bufs=1) as xtp,
            tc.tile_pool(name="g", bufs=1) as gp_,
            tc.tile_pool(name="a", bufs=1) as ap_,
            tc.tile_pool(name="ctx", bufs=2) as cxp,
            tc.tile_pool(name="osb", bufs=3) as osp,
            tc.tile_pool(name="ps_g", bufs=1, space="PSUM") as psg,
            tc.tile_pool(name="ps_t", bufs=2, space="PSUM") as pst,
            tc.tile_pool(name="ps_o", bufs=2, space="PSUM") as pso,
        ):
            ident = constp.tile([128, 128], BF, tag="ident")
            make_identity(nc, ident[:])
            muI = constp.tile([128, 128], F32, tag="muI")
            nc.gpsimd.memset(muI[:], 0.0)
            nc.gpsimd.affine_select(
                out=muI[:], in_=muI[:],
                compare_op=mybir.AluOpType.not_equal, fill=MU,
                base=0, pattern=[[-1, 128]], channel_multiplier=1,
            )

            # ---- phase 0: weights + fp32 TT = mu * Wv^T Wk per head ----
            whi = []   # bf16 weights [128, CB, 2C]
            tts = []   # fp32 mu*(Wv^T Wk), [64, C] (heads along free)
            for s in range(2):
                wf = wfp.tile([128, CB, 2 * C], F32, tag="wf")
                src = w_d[s][:, :].rearrange("(a p) m -> p a m", p=128)
                nc.sync.dma_start(out=wf[:], in_=src)
                wh = wp.tile([128, CB, 2 * C], BF, tag="w")
                for a in range(CB):
                    nc.scalar.copy(wh[:, a, :], wf[:, a, :])
                whi.append(wh)

                ttp = pso.tile([64, C], F32, tag="pbig", name=f"ttp{s}")
                for h in range(H):
                    for a in range(CB):
                        nc.tensor.matmul(
                            ttp[:, 64 * h:64 * (h + 1)],
                            lhsT=wf[:, a, C + 64 * h:C + 64 * (h + 1)],
                            rhs=wf[:, a, 64 * h:64 * (h + 1)],
                            start=(a == 0), stop=(a == CB - 1),
                        )
                tt = cxp.tile([64, C], F32, tag="tts")
                nc.scalar.mul(tt[:], ttp[:], MU)
                tts.append(tt)

            def copy_alt(i, out, in_):
                if i % 2 == 0:
                    nc.scalar.copy(out, in_)
                else:
                    nc.vector.tensor_copy(out, in_)

            xts, cbds = [], []

            def gram_and_ctx(s):
                """Load x_s (streamed), Gram, xT, then ctx_s -> Cbd tiles."""
                # Gram accumulators: G[m] covers rows c in block m,
                # cols c' in [128m, 512) (upper triangle by tile).
                gps = []
                for m in range(CB):
                    gt_ = psg.tile([128, C - 128 * m], F32, tag=f"gp{m}",
                                   name=f"gp{m}_{s}")
                    gps.append(gt_[:])
                xt = xtp.tile([128, CB, N], BF, tag=f"xt{s}")
                xts.append(xt)
                for r in range(NT // 4):
                    # fp32 DRAM -> bf16 SBUF (SWDGE cast), 1 MiB fp32 chunk.
                    xc = xp.tile([128, 4, C], BF, tag="xc", name=f"xc{s}_{r}")
                    src = x_d[s][512 * r:512 * (r + 1), :].rearrange(
                        "(t p) c -> p t c", p=128)
                    nc.gpsimd.dma_start(out=xc[:], in_=src)
                    for tt_ in range(4):
                        t = 4 * r + tt_
                        for m in range(CB):
                            nc.tensor.matmul(
                                gps[m],
                                lhsT=xc[:, tt_, 128 * m:128 * (m + 1)],
                                rhs=xc[:, tt_, 128 * m:],
                                start=(t == 0), stop=(t == NT - 1),
                            )
                        tp4 = pst.tile([128, CB, 128], BF, tag="tps",
                                       name=f"tp4_{s}_{t}")
                        for cb in range(CB):
                            nc.tensor.transpose(
                                tp4[:, cb, :], xc[:, tt_, 128 * cb:128 * (cb + 1)],
                                ident[:])
                        copy_alt(t, xt[:, :, 128 * t:128 * (t + 1)], tp4[:])

                # Gc = G - mu*I in bf16; gsb[:, a, f] = G[128a+p, f]
                gsb = gp_.tile([128, CB, C], BF, tag=f"g{s}")
                for m in range(CB):
                    nc.vector.tensor_sub(
                        gsb[:, m, 128 * m:128 * (m + 1)],
                        gps[m][:, 0:128], muI[:])
                    if m < CB - 1:
                        copy_alt(m, gsb[:, m, 128 * (m + 1):], gps[m][:, 128:])
                # lower-triangle tiles by PE transpose
                low = {}
                gtr = gp_.tile([128, 6, 128], BF, tag=f"gt{s}")
                idx = 0
                for a2 in range(CB):
                    for b2 in range(a2):
                        tp = pst.tile([128, 128], BF, tag="tps")
                        nc.tensor.transpose(
                            tp[:], gsb[:, b2, 128 * a2:128 * (a2 + 1)], ident[:])
                        nc.scalar.copy(gtr[:, idx, :], tp[:])
                        low[(a2, b2)] = idx
                        idx += 1

                def g_tile(a2, b2):
                    if b2 >= a2:
                        return gsb[:, a2, 128 * b2:128 * (b2 + 1)]
                    return gtr[:, low[(a2, b2)], :]

                # A = Gc^T-tiles @ Wv  (Ac[c,e]); hi/lo bf16 split
                wh = whi[s]
                ahi = ap_.tile([128, CB, C], BF, tag=f"ah{s}")
                alo = ap_.tile([128, CB, C], BF, tag=f"al{s}")
                for b2 in range(CB):
                    apx = pso.tile([128, C], F32, tag="pbig", name=f"apx{s}_{b2}")
                    for a2 in range(CB):
                        nc.tensor.matmul(
                            apx[:], lhsT=g_tile(a2, b2), rhs=wh[:, a2, C:],
                            start=(a2 == 0), stop=(a2 == CB - 1))
                    nc.scalar.copy(ahi[:, b2, :], apx[:])
                    nc.vector.tensor_sub(alo[:, b2, :], apx[:], ahi[:, b2, :])

                # ctxpreT[e,d] per head + TT, exp, normalize
                ctp = pso.tile([64, C], F32, tag="pbig", name=f"ctp{s}")
                for h in range(H):
                    sl = slice(64 * h, 64 * (h + 1))
                    for a2 in range(CB):
                        nc.tensor.matmul(
                            ctp[:, sl], lhsT=ahi[:, a2, sl], rhs=wh[:, a2, sl],
                            start=(a2 == 0), stop=False)
                    for a2 in range(CB):
                        nc.tensor.matmul(
                            ctp[:, sl], lhsT=alo[:, a2, sl], rhs=wh[:, a2, sl],
                            start=False, stop=(a2 == CB - 1))
                comb = cxp.tile([64, C], F32, tag="comb")
                esb = cxp.tile([64, C], F32, tag="esb")
                ssum = cxp.tile([64, H], F32, tag="ssum")
                rsum = cxp.tile([64, H], F32, tag="rsum")
                nmax = cxp.tile([64, H], F32, tag="nmax")
                nmaxs = cxp.tile([64, H], F32, tag="nmaxs")
                ctxts = cxp.tile([64, C], BF, tag="ctxts")
                for h in range(H):
                    sl = slice(64 * h, 64 * (h + 1))
                    nc.vector.tensor_add(comb[:, sl], ctp[:, sl], tts[s][:, sl])
                    nc.vector.reduce_max(
                        nmax[:, h:h + 1], comb[:, sl],
                        axis=mybir.AxisListType.X, negate=True)
                nc.scalar.mul(nmaxs[:], nmax[:], SCALE)
                for h in range(H):
                    sl = slice(64 * h, 64 * (h + 1))
                    nc.scalar.activation(
                        esb[:, sl], comb[:, sl], AF.Exp, scale=SCALE,
                        bias=nmaxs[:, h:h + 1], accum_out=ssum[:, h:h + 1])
                nc.vector.reciprocal(rsum[:], ssum[:])
                for h in range(H):
                    sl = slice(64 * h, 64 * (h + 1))
                    nc.vector.tensor_scalar_mul(
                        ctxts[:, sl], esb[:, sl], rsum[:, h:h + 1])
                # transpose each head -> natural ctx, pack block-diag tiles
                cbd = cxp.tile([128, CB, 128], BF, tag=f"cbd{s}")
                nc.vector.memset(cbd[:], 0.0)
                for h in range(H):
                    tp = pst.tile([128, 128], BF, tag="tps")
                    t2, r2 = h // 2, (h % 2) * 64
                    nc.tensor.transpose(
                        tp[r2:r2 + 64, r2:r2 + 64],
                        ctxts[:, 64 * h:64 * (h + 1)],
                        ident[0:64, 0:64], tile_position=(0, r2))
                    nc.scalar.copy(
                        cbd[r2:r2 + 64, t2, r2:r2 + 64], tp[r2:r2 + 64, r2:r2 + 64])
                cbds.append(cbd)

            def out_stage(s):
                """o_s = x_s @ blockdiag(ctx_other)."""
                xt, cbd = xts[s], cbds[1 - s]
                for r in range(NT // 4):
                    ob = osp.tile([128, 4, C], BF, tag="ob")
                    for tt_ in range(4):
                        t = 4 * r + tt_
                        op = psg.tile([128, C], F32, tag=f"gp{t % 2}",
                                      name=f"op{s}_{t}")
                        for cb in range(CB):
                            nc.tensor.matmul(
                                op[:, 128 * cb:128 * (cb + 1)],
                                lhsT=xt[:, cb, 128 * t:128 * (t + 1)],
                                rhs=cbd[:, cb, :],
                                start=True, stop=True)
                        copy_alt(t, ob[:, tt_, :], op[:])
                    dst = o_d[s][512 * r:512 * (r + 1), :].rearrange(
                        "(t p) c -> p t c", p=128)
                    nc.sync.dma_start(out=dst, in_=ob[:])

            gram_and_ctx(0)      # x1 load, G1, xT1, ctx1
            gram_and_ctx(1)      # x2 load, G2, xT2, ctx2
            tc.strict_bb_all_engine_barrier()
            out_stage(1)         # o2 = x2 @ Cbd1
            out_stage(0)         # o1 = x1 @ Cbd2
    nc.compile()
    return nc


_NC = None


def kernel(x1, x2, W_kv1, W_kv2):
    global _NC
    if _NC is None:
        _NC = build()
    x1 = np.ascontiguousarray(x1, dtype=np.float32)
    x2 = np.ascontiguousarray(x2, dtype=np.float32)
    W1 = np.ascontiguousarray(W_kv1, dtype=np.float32)
    W2 = np.ascontiguousarray(W_kv2, dtype=np.float32)
    in_maps = [
        {"x1": x1[b], "x2": x2[b], "W_kv1": W1, "W_kv2": W2} for b in range(B)
    ]
    res = run_bass_kernel_spmd(_NC, in_maps, core_ids=list(range(B)))
    o1 = np.stack([res.results[b]["o1"].astype(np.float32) for b in range(B)])
    o2 = np.stack([res.results[b]["o2"].astype(np.float32) for b in range(B)])
    return o1, o2



# revision 7
# speedup vs baseline: 1.2450x; 1.2450x over previous
"""Trainium2 Bass kernel for nn_CrossAttention_249108103802.

Math (per batch b, one NeuronCore; 8 cores data-parallel over B=8):
  q_s   = heads(x_s)                   (column slices of x_s)
  k,v   = x_s @ W_s  split per head    -> never materialized; instead:
  ctx_s = softmax_d(scale * k^T v)     via Gram trick:
          k_h^T v_h = Wk_h^T (x^T x) Wv_h
  o1    = q1 @ blockdiag(ctx2), o2 = q2 @ blockdiag(ctx1)

Precision: bf16 operands on the PE with fp32 PSUM accumulation. The Gram
matrix is split G = Gc + mu*I (mu = N) so Gc fits bf16; the mu*Wv term is
re-added inside the A = G @ Wv matmul via a bf16 mu*I stationary. A gets a
hi/lo bf16 split before ctp = A^T Wk. SCALE is folded into the Wk cast, and
softmax is stabilized with a per-row max (negated reduce as the exp bias).

Schedule: x1 streams in (SWDGE fp32->bf16 cast) feeding Gram1 + PE
transposes immediately; W loads ride the HWDGE queue concurrently; then
Gram2, ctx1, A2/ctp2, out2 (overlapping softmax2), out1. Out-stage matmuls
rotate 4 PSUM banks with evacuation split across scalar/vector.
"""
import sys

sys.path.insert(0, "/opt/trn_rl_repo")

import numpy as np

import concourse.bass as bass
import concourse.mybir as mybir
import concourse.tile as tile
from concourse import bacc
from concourse.bass_utils import run_bass_kernel_spmd
from concourse.masks import make_identity

B, N, C, H = 8, 4096, 512, 8
HD = C // H                    # 64
SCALE = HD ** -0.5             # 1/8
MU = float(N)                  # expected Gram diagonal
NT = N // 128                  # 32 row tiles
CB = C // 128                  # 4 feature blocks
BF = mybir.dt.bfloat16
F32 = mybir.dt.float32
AF = mybir.ActivationFunctionType
AX = mybir.AxisListType


def build():
    nc = bacc.Bacc("TRN2", target_bir_lowering=False, debug=False, num_devices=8)
    x_d = [nc.declare_dram_parameter("x1", [N, C], F32, isOutput=False),
           nc.declare_dram_parameter("x2", [N, C], F32, isOutput=False)]
    w_d = [nc.declare_dram_parameter("W_kv1", [C, 2 * C], F32, isOutput=False),
           nc.declare_dram_parameter("W_kv2", [C, 2 * C], F32, isOutput=False)]
    o_d = [nc.declare_dram_parameter("o1", [N, C], BF, isOutput=True),
           nc.declare_dram_parameter("o2", [N, C], BF, isOutput=True)]

    with tile.TileContext(nc) as tc:
        with (
            tc.tile_pool(name="const", bufs=1) as constp,
            tc.tile_pool(name="wf", bufs=2) as wfp,
            tc.tile_pool(name="w", bufs=1) as wp,
            tc.tile_pool(name="x", bufs=4) as xp,
            tc.tile_pool(name="xt", bufs=1) as xtp,
            tc.tile_pool(name="g", bufs=1) as gp_,
            tc.tile_pool(name="a", bufs=1) as ap_,
            tc.tile_pool(name="ctx", bufs=2) as cxp,
            tc.tile_pool(name="osb", bufs=3) as osp,
            tc.tile_pool(name="ps_g", bufs=1, space="PSUM") as psg,
            tc.tile_pool(name="ps_t", bufs=2, space="PSUM") as pst,
            tc.tile_pool(name="ps_o", bufs=2, space="PSUM") as pso,
        ):
            ident = constp.tile([128, 128], BF, tag="ident")
            make_identity(nc, ident[:])
            # fp32 mu*I for the Gram-diagonal subtraction
            muI = constp.tile([128, 128], F32, tag="muI")
            nc.gpsimd.memset(muI[:], 0.0)
            nc.gpsimd.affine_select(
                out=muI[:], in_=muI[:],
                compare_op=mybir.AluOpType.not_equal, fill=MU,
                base=0, pattern=[[-1, 128]], channel_multiplier=1,
            )
            # bf16 mu*I stationary for re-adding mu*Wv inside the A matmul
            muIb = constp.tile([128, 128], BF, tag="muIb")
            nc.gpsimd.memset(muIb[:], 0.0)
            nc.gpsimd.affine_select(
                out=muIb[:], in_=muIb[:],
                compare_op=mybir.AluOpType.not_equal, fill=MU,
                base=0, pattern=[[-1, 128]], channel_multiplier=1,
            )

            def copy_alt(i, out, in_):
                if i % 2 == 0:
                    nc.scalar.copy(out, in_)
                else:
                    nc.vector.tensor_copy(out, in_)

            xts, gsbs, gtrs, lows, cbds = [], [], [], [], []

            # ---- load x_s, Gram accumulation, and PE transposes ----
            def gram_stage(s):
                gps = []
                for m in range(CB):
                    gt_ = psg.tile([128, 512], F32, tag=f"g{m}",
                                   name=f"gp{m}_{s}")
                    gps.append(gt_)
                xt = xtp.tile([128, CB, N], BF, tag=f"xt{s}")
                xts.append(xt)
                for r in range(NT // 4):
                    # fp32 DRAM -> bf16 SBUF (SWDGE cast), 1 MiB fp32 chunk.
                    xc = xp.tile([128, 4, C], BF, tag="xc", name=f"xc{s}_{r}")
                    src = x_d[s][512 * r:512 * (r + 1), :].rearrange(
                        "(t p) c -> p t c", p=128)
                    nc.gpsimd.dma_start(out=xc[:], in_=src)
                    for tt_ in range(4):
                        t = 4 * r + tt_
                        for m in range(CB):
                            nc.tensor.matmul(
                                gps[m][:, :C - 128 * m],
                                lhsT=xc[:, tt_, 128 * m:128 * (m + 1)],
                                rhs=xc[:, tt_, 128 * m:],
                                start=(t == 0), stop=(t == NT - 1),
                            )
                        tp4 = pst.tile([128, CB, 128], BF, tag="tps",
                                       name=f"tp4_{s}_{t}")
                        for cb in range(CB):
                            nc.tensor.transpose(
                                tp4[:, cb, :], xc[:, tt_, 128 * cb:128 * (cb + 1)],
                                ident[:])
                        copy_alt(t, xt[:, :, 128 * t:128 * (t + 1)], tp4[:])

                # Gc = G - mu*I in bf16; gsb[:, a, f] = G[128a+p, f]
                gsb = gp_.tile([128, CB, C], BF, tag=f"g{s}")
                for m in range(CB):
                    nc.vector.tensor_sub(
                        gsb[:, m, 128 * m:128 * (m + 1)],
                        gps[m][:, 0:128], muI[:])
                    if m < CB - 1:
                        copy_alt(m, gsb[:, m, 128 * (m + 1):],
                                 gps[m][:, 128:C - 128 * m])
                # lower-triangle tiles by PE transpose
                low = {}
                gtr = gp_.tile([128, 6, 128], BF, tag=f"gt{s}")
                idx = 0
                for a2 in range(CB):
                    for b2 in range(a2):
                        tp = pst.tile([128, 128], BF, tag="tps")
                        nc.tensor.transpose(
                            tp[:], gsb[:, b2, 128 * a2:128 * (a2 + 1)], ident[:])
                        nc.scalar.copy(gtr[:, idx, :], tp[:])
                        low[(a2, b2)] = idx
                        idx += 1
                gsbs.append(gsb)
                gtrs.append(gtr)
                lows.append(low)

            # ---- weights: load + cast (k-half pre-scaled by SCALE) ----
            whi = []

            wfs = []

            def weight_load():
                for s in range(2):
                    wf = wfp.tile([128, CB, 2 * C], F32, tag="wf",
                                  name=f"wf{s}")
                    src = w_d[s][:, :].rearrange("(a p) m -> p a m", p=128)
                    nc.sync.dma_start(out=wf[:], in_=src)
                    wfs.append(wf)
                    wh = wp.tile([128, CB, 2 * C], BF, tag=f"w{s}",
                                 name=f"wh{s}")
                    whi.append(wh)

            def weight_cast(s):
                # v-half plain cast on scalar, k-half scaled (SCALE = 2^-3,
                # exact in bf16) on vector; scheduled into engine slack.
                wf, wh = wfs[s], whi[s]
                for a in range(CB):
                    nc.scalar.copy(wh[:, a, C:], wf[:, a, C:])
                    nc.vector.tensor_scalar_mul(
                        wh[:, a, 0:C], wf[:, a, 0:C], SCALE)

            # ---- ctx_s: A = G @ Wv (mu folded in), ctp = A^T Wk_scaled ----
            def ctx_mm_stage(s):
                gsb, gtr, low = gsbs[s], gtrs[s], lows[s]

                def g_tile(a2, b2):
                    if b2 >= a2:
                        return gsb[:, a2, 128 * b2:128 * (b2 + 1)]
                    return gtr[:, low[(a2, b2)], :]

                wh = whi[s]
                ahi = ap_.tile([128, CB, C], BF, tag="ah", name=f"ah{s}")
                alo = ap_.tile([128, CB, C], BF, tag="al", name=f"al{s}")
                for b2 in range(CB):
                    apx = pso.tile([128, C], F32, tag="pbig", name=f"apx{s}_{b2}")
                    for a2 in range(CB):
                        nc.tensor.matmul(
                            apx[:], lhsT=g_tile(a2, b2), rhs=wh[:, a2, C:],
                            start=(a2 == 0), stop=False)
                    # += mu * Wv rows of block b2  (G = Gc + mu*I)
                    nc.tensor.matmul(
                        apx[:], lhsT=muIb[:], rhs=wh[:, b2, C:],
                        start=False, stop=True)
                    nc.scalar.copy(ahi[:, b2, :], apx[:])
                    nc.vector.tensor_sub(alo[:, b2, :], apx[:], ahi[:, b2, :])

                # ctp[e, d] per head = (A^T Wk_scaled), hi + lo
                ctp = pso.tile([64, C], F32, tag="pbig", name=f"ctp{s}")
                for h in range(H):
                    sl = slice(64 * h, 64 * (h + 1))
                    for a2 in range(CB):
                        nc.tensor.matmul(
                            ctp[:, sl], lhsT=ahi[:, a2, sl], rhs=wh[:, a2, sl],
                            start=(a2 == 0), stop=False)
                    for a2 in range(CB):
                        nc.tensor.matmul(
                            ctp[:, sl], lhsT=alo[:, a2, sl], rhs=wh[:, a2, sl],
                            start=False, stop=(a2 == CB - 1))
                return ctp

            # ---- softmax over d (free axis) + block-diag ctx tiles ----
            def softmax_stage(s, ctp):
                esb = cxp.tile([64, C], F32, tag="esb")
                ssum = cxp.tile([64, H], F32, tag="ssum")
                rsum = cxp.tile([64, H], F32, tag="rsum")
                nmax = cxp.tile([64, H], F32, tag="nmax")
                ctxts = cxp.tile([64, C], BF, tag="ctxts")
                for h in range(H):
                    sl = slice(64 * h, 64 * (h + 1))
                    nc.vector.reduce_max(
                        nmax[:, h:h + 1], ctp[:, sl], axis=AX.X, negate=True)
                for h in range(H):
                    sl = slice(64 * h, 64 * (h + 1))
                    nc.scalar.activation(
                        esb[:, sl], ctp[:, sl], AF.Exp,
                        bias=nmax[:, h:h + 1], accum_out=ssum[:, h:h + 1])
                nc.vector.reciprocal(rsum[:], ssum[:])
                for h in range(H):
                    sl = slice(64 * h, 64 * (h + 1))
                    nc.vector.tensor_scalar_mul(
                        ctxts[:, sl], esb[:, sl], rsum[:, h:h + 1])
                # 2 heads per PE transpose -> natural ctx block-diag tiles
                cbd = cxp.tile([128, CB, 128], BF, tag=f"cbd{s}")
                nc.vector.memset(cbd[:], 0.0)
                for t2 in range(CB):
                    tp = pst.tile([128, 128], BF, tag="tps")
                    nc.tensor.transpose(
                        tp[:, 0:64], ctxts[:, 128 * t2:128 * (t2 + 1)],
                        ident[0:64, 0:64])
                    nc.scalar.copy(cbd[0:64, t2, 0:64], tp[0:64, 0:64])
                    nc.scalar.copy(cbd[64:128, t2, 64:128], tp[64:128, 0:64])
                cbds.append(cbd)

            # ---- o_s = x_s @ blockdiag(ctx_other), 4-bank rotation ----
            def out_stage(s):
                xt, cbd = xts[s], cbds[1 - s]
                for r in range(NT // 4):
                    ob = osp.tile([128, 4, C], BF, tag="ob")
                    for pp in range(2):       # tile pairs within the group
                        ops = []
                        for q in range(2):
                            t = 4 * r + 2 * pp + q
                            bank = 2 * pp + q
                            op = psg.tile([128, 512], F32, tag=f"g{bank}",
                                          name=f"op{s}_{t}")
                            ops.append(op)
                        for cb in range(CB):  # alternate banks per MM
                            for q in range(2):
                                t = 4 * r + 2 * pp + q
                                nc.tensor.matmul(
                                    ops[q][:, 128 * cb:128 * (cb + 1)],
                                    lhsT=xt[:, cb, 128 * t:128 * (t + 1)],
                                    rhs=cbd[:, cb, :],
                                    start=True, stop=True)
                        for q in range(2):
                            copy_alt(q, ob[:, 2 * pp + q, :], ops[q][:])
                    dst = o_d[s][512 * r:512 * (r + 1), :].rearrange(
                        "(t p) c -> p t c", p=128)
                    nc.sync.dma_start(out=dst, in_=ob[:])

            weight_load()              # W loads ride HWDGE concurrently
            gram_stage(0)              # x1 load, G1, xT1
            weight_cast(0)
            gram_stage(1)              # x2 load, G2, xT2
            weight_cast(1)
            ctp1 = ctx_mm_stage(0)
            softmax_stage(0, ctp1)     # -> cbds[0]
            ctp2 = ctx_mm_stage(1)
            out_stage(1)               # o2 = x2 @ Cbd1 (overlaps softmax2)
            softmax_stage(1, ctp2)     # -> cbds[1]
            out_stage(0)               # o1 = x1 @ Cbd2
    nc.compile()
    return nc


_NC = None


def kernel(x1, x2, W_kv1, W_kv2):
    global _NC
    if _NC is None:
        _NC = build()
    x1 = np.ascontiguousarray(x1, dtype=np.float32)
    x2 = np.ascontiguousarray(x2, dtype=np.float32)
    W1 = np.ascontiguousarray(W_kv1, dtype=np.float32)
    W2 = np.ascontiguousarray(W_kv2, dtype=np.float32)
    in_maps = [
        {"x1": x1[b], "x2": x2[b], "W_kv1": W1, "W_kv2": W2} for b in range(B)
    ]
    res = run_bass_kernel_spmd(_NC, in_maps, core_ids=list(range(B)))
    o1 = np.stack([res.results[b]["o1"].astype(np.float32) for b in range(B)])
    o2 = np.stack([res.results[b]["o2"].astype(np.float32) for b in range(B)])
    return o1, o2
